# revision 1
# baseline (speedup 1.0000x reference)
"""nn_MultiHeadAttention_59253368815813 on 8 TRN2 NeuronCores.

The reference module is bug-faithful to its original nn.Module in two ways
that together collapse the computation:

  1. ``o = jnp.einsum('bhtl,bthd->bhtd', A, v)`` indexes ``v`` by the QUERY
     position ``t``, not the key position ``l``. ``l`` therefore only sums
     over the softmax weights, which sum to exactly 1 per row:
     ``o[b,h,t,d] == v[b,t,h,d]``. Q, K, the mask and the softmax never
     influence the output (verified vs the reference to 4e-7 rel).
  2. ``o.reshape(b, T, d)`` with no transpose scrambles (head, token) so the
     reshaped activation row tj = 128*h + s is the concatenation over
     m=0..15 of v[b, 16*s+m, h, :].

So the exact computation is  out = scramble(x @ Wv) @ Wo.T,  and the
scramble makes output rows depend on one head only. Sharding: core c owns
heads {2c, 2c+1}, i.e. Wv columns [128c, 128c+128) and output rows
[256c, 256c+256) of each batch; the host concatenates the row slabs.
No cross-core reduction needed.

Per core (fp32r matmuls, fp32 PSUM):
  vT[128ch, u] = Wv_slice^T @ x^T   where the host feeds x^T with tokens
  permuted to u = m*128 + r (t = 16r + m), so the reshape scramble becomes
  contiguous: the PSUM evacuation writes vt2[64*(m%2)+di, (m//2)*128+r] and
  the output projection is 8 accumulating K=128 matmuls per output tile:
  out[128h + r, n] = sum_m2 vt2_chunk(m2)^T @ WoT[128*m2:128*m2+128, n].
"""

import sys
import types

import numpy as np

_TRN_REPO = "/opt/trn_rl_repo"
if _TRN_REPO not in sys.path:
    sys.path.insert(0, _TRN_REPO)


def _install_ntff_shim():
    """antenv.axon_hooks is absent in this container; provide it so
    BASS_TRACE=1 profiling works. No-op if the real module exists."""
    try:
        import antenv  # noqa: F401
    except ImportError:
        return
    if "antenv.axon_hooks" in sys.modules:
        return
    try:
        import antenv.axon_hooks  # noqa: F401
        return
    except ImportError:
        pass
    m = types.ModuleType("antenv.axon_hooks")
    m._hook = None
    m.set_axon_ntff_profile_hook = lambda h: setattr(m, "_hook", h)
    m.get_axon_ntff_profile_hook = lambda: m._hook
    sys.modules["antenv.axon_hooks"] = m
    try:
        from trn_agent_boot.trn_boot import _ntff_profile_via_ctypes

        hook = _ntff_profile_via_ctypes("/opt/axon/libaxon_pjrt.so")
        if hook is not None:
            m.set_axon_ntff_profile_hook(hook)
    except Exception:
        pass


_install_ntff_shim()

import concourse.mybir as mybir  # noqa: E402
import concourse.tile as tile  # noqa: E402
from concourse import bacc  # noqa: E402
from concourse.bass_utils import run_bass_kernel_spmd  # noqa: E402

F32 = mybir.dt.float32
F32R = mybir.dt.float32r

B = 2
T = 2048
D = 1024
NCORES = 8
GT = B * T          # 4096
NG = GT // 512      # 8 global 512-token chunks
NDCH = D // 128     # 8 contraction chunks for the projection

_CACHED = None
LAST_RESULTS = None


def _build_module():
    nc = bacc.Bacc("TRN2", target_bir_lowering=False, debug=False,
                   num_devices=NCORES)

    xT_d = nc.dram_tensor("xT", [D, GT], F32R, kind="ExternalInput").ap()
    wv_d = nc.dram_tensor("wv", [128, NDCH, 128], F32R,
                          kind="ExternalInput").ap()
    wo_d = nc.dram_tensor("woT", [128, 8, D], F32R,
                          kind="ExternalInput").ap()
    out_d = nc.dram_tensor("out", [B, 256, D], F32, kind="ExternalOutput").ap()

    with tile.TileContext(nc) as tc:
        _emit(nc, tc, xT_d, wv_d, wo_d, out_d)
    nc.compile()
    return nc


def _emit(nc, tc, xT_d, wv_d, wo_d, out_d):
    from contextlib import ExitStack

    ctx = ExitStack()
    with ctx:
        wpool = ctx.enter_context(tc.tile_pool(name="w", bufs=1))
        xtp = ctx.enter_context(tc.tile_pool(name="xt", bufs=4))
        vtp = ctx.enter_context(tc.tile_pool(name="vt", bufs=1))
        outp = ctx.enter_context(tc.tile_pool(name="outsb", bufs=4))
        ps_p = ctx.enter_context(tc.tile_pool(name="ps_p", bufs=4, space="PSUM"))
        ps_w = ctx.enter_context(tc.tile_pool(name="ps_w", bufs=3, space="PSUM"))

        # weights ride the ACT HWDGE ring; activations the SP ring (parallel)
        wv_sb = wpool.tile([128, NDCH, 128], F32R, tag="wv")
        nc.scalar.dma_start(wv_sb[:], wv_d)
        wo_sb = wpool.tile([128, 8, D], F32R, tag="wo")

        # vt2[h][64*(m%2)+di, b*1024 + (m//2)*128 + r] = v[b, t=16r+m, 64h+di]
        vt = [vtp.tile([128, GT // 2], F32R, tag=f"vt{h}", name=f"vt{h}")
              for h in range(2)]

        def proj_half(half, before_j=None):
            """v^T for one 2048-token half (= one batch). before_j(j) lets the
            caller interleave other PE work between the 2MB-chunk groups."""
            pss = [ps_p.tile([128, 512], F32, tag="proj",
                             name=f"psp{half}_{q}") for q in range(4)]
            for j in range(4):
                xt = xtp.tile([128, 2, 2048], F32R, tag="xt",
                              name=f"xt{half}_{j}")
                nc.sync.dma_start(
                    xt[:], xT_d[j * 256:(j + 1) * 256,
                                half * 2048:(half + 1) * 2048]
                    .rearrange("(ko ki) t -> ki ko t", ki=128))
                if before_j is not None:
                    before_j(j)
                for kk in range(2):
                    dch = 2 * j + kk
                    for q in range(4):
                        nc.tensor.matmul(pss[q][:], wv_sb[:, dch, :],
                                         xt[:, kk, q * 512:(q + 1) * 512],
                                         start=(dch == 0),
                                         stop=(dch == NDCH - 1))

            for q in range(4):
                for h in range(2):
                    for mm in range(4):
                        m = q * 4 + mm
                        j, m2 = m % 2, m // 2
                        nc.vector.tensor_copy(
                            vt[h][64 * j:64 * j + 64,
                                  half * 1024 + m2 * 128:
                                  half * 1024 + (m2 + 1) * 128],
                            pss[q][64 * h:64 * h + 64,
                                   mm * 128:(mm + 1) * 128])

        def wo_block(b, h, nch):
            """Output rows [128h, 128h+128) of batch b, cols [512nch, +512)."""
            ps = ps_w.tile([128, 512], F32, tag="wo", name=f"psw{b}_{h}_{nch}")
            for m2 in range(8):
                lhs = vt[h][:, b * 1024 + m2 * 128:b * 1024 + (m2 + 1) * 128]
                nc.tensor.matmul(ps[:], lhs,
                                 wo_sb[:, m2, nch * 512:(nch + 1) * 512],
                                 start=(m2 == 0), stop=(m2 == 7))
            ob = outp.tile([128, 512], F32, tag="ob", name=f"ob{b}_{h}_{nch}")
            nc.vector.tensor_copy(ob[:], ps[:])
            nc.scalar.dma_start(
                out_d[b, 128 * h:128 * h + 128,
                      nch * 512:(nch + 1) * 512], ob[:])

        proj_half(0)
        # woT rides the sync ring between the two xt halves: it doesn't steal
        # bandwidth from xt(half0, 0) and still lands before the first wo_block
        nc.sync.dma_start(wo_sb[:], wo_d)
        # during half-1's DMA stream, fill PE gaps with batch-0 out-proj
        proj_half(1, before_j=lambda j: wo_block(0, j // 2, j % 2))
        for h in range(2):
            for nch in range(2):
                wo_block(1, h, nch)


def _get_module():
    global _CACHED
    if _CACHED is None:
        _CACHED = _build_module()
    return _CACHED


def _round_f32r(a):
    """Round fp32 to the fp32r grid (RNE at 11 mantissa bits) — verified
    bit-identical to the hardware fp32->fp32r cast."""
    b = np.ascontiguousarray(a, np.float32).view(np.uint32).astype(np.uint64)
    lsb = (b >> 12) & 1
    out = (b + 0x7FF + lsb) & np.uint64(0xFFFFF000)
    return out.astype(np.uint32).view(np.float32)


def kernel(x, mask, Wq, Wk, Wv, Wo):
    global LAST_RESULTS
    x = np.asarray(x, dtype=np.float32)
    Wv = np.asarray(Wv, dtype=np.float32)
    Wo = np.asarray(Wo, dtype=np.float32)

    b, t, d = x.shape
    assert (b, t, d) == (B, T, D), (b, t, d)

    # x^T with tokens permuted to u = m*128 + r  (original t = 16r + m)
    xT = x.transpose(2, 0, 1).reshape(D, B, 128, 16)
    xT = _round_f32r(xT.swapaxes(2, 3).reshape(D, GT))
    # woT[p, m2, n] = Wo.T[128*m2 + p, n]
    woT = _round_f32r(Wo.T.reshape(8, 128, D).transpose(1, 0, 2))
    wv_r = _round_f32r(Wv)

    in_maps = []
    for c in range(NCORES):
        wv_c = wv_r[:, 128 * c:128 * c + 128]  # [1024, 128]
        wv_c = np.ascontiguousarray(
            wv_c.reshape(NDCH, 128, 128).transpose(1, 0, 2))
        in_maps.append({
            "xT": xT,
            "woT": woT,
            "wv": wv_c,
        })

    nc = _get_module()
    res = run_bass_kernel_spmd(nc, in_maps, list(range(NCORES)))
    LAST_RESULTS = res
    out = np.concatenate([res.results[c]["out"] for c in range(NCORES)],
                         axis=1)
    return np.ascontiguousarray(out.astype(np.float32))



# revision 5
# speedup vs baseline: 1.7513x; 1.7513x over previous
"""nn_MultiHeadAttention_59253368815813 on 8 TRN2 NeuronCores.

The reference module is bug-faithful to its original nn.Module in two ways
that together collapse the computation:

  1. ``o = jnp.einsum('bhtl,bthd->bhtd', A, v)`` indexes ``v`` by the QUERY
     position ``t``, not the key position ``l``. ``l`` therefore only sums
     over the softmax weights, which sum to exactly 1 per row:
     ``o[b,h,t,d] == v[b,t,h,d]``. Q, K, the mask and the softmax never
     influence the output.
  2. ``o.reshape(b, T, d)`` with no transpose scrambles (head, token) so the
     reshaped activation row tj = 128*h + s is the concatenation over
     m=0..15 of v[b, 16*s+m, h, :].

So the exact computation is  out = scramble(x @ Wv) @ Wo.T,  and the
scramble makes output rows depend on one head only.

Sharding: 2 batches x 4 head-groups. Core c = (b=c//4, g=c%4) owns batch b
and heads {4g..4g+3} = Wv columns [256g, 256g+256) and output rows
[512g, 512g+512) of batch b. Each core loads only its batch's x (4.2MB in
bf16) instead of all of x, which is what made the previous version
DMA-bound (23.3MB/core at a shared ~360GB/s).

Per core, all in bf16 (PE runs bf16 at 1 cycle/row like f32r, but DMA
halves; quantization error ~2e-3 << the 2e-2 gate):
  stream x^T (tokens permuted to u = 128m + s, t = 16s + m) in 8 blocks of
  256 tokens; v-proj psum [128,256] per head-pair chases the stream; the
  reshape scramble happens in the psum->SBUF evacuation copies (spread over
  DVE/Pool/Act engines); output-projection columns [0,512) accumulate
  interleaved with the stream (chunk k uses only v tokens of block k);
  columns [512,1024) run as a second pass after the stream, overlapping the
  output DMAs.
"""

import sys
import types

import numpy as np

_TRN_REPO = "/opt/trn_rl_repo"
if _TRN_REPO not in sys.path:
    sys.path.insert(0, _TRN_REPO)


def _install_ntff_shim():
    """antenv.axon_hooks is absent in this container; provide it so
    BASS_TRACE=1 profiling works. No-op if the real module exists."""
    try:
        import antenv  # noqa: F401
    except ImportError:
        return
    if "antenv.axon_hooks" in sys.modules:
        return
    try:
        import antenv.axon_hooks  # noqa: F401
        return
    except ImportError:
        pass
    m = types.ModuleType("antenv.axon_hooks")
    m._hook = None
    m.set_axon_ntff_profile_hook = lambda h: setattr(m, "_hook", h)
    m.get_axon_ntff_profile_hook = lambda: m._hook
    sys.modules["antenv.axon_hooks"] = m
    try:
        from trn_agent_boot.trn_boot import _ntff_profile_via_ctypes

        hook = _ntff_profile_via_ctypes("/opt/axon/libaxon_pjrt.so")
        if hook is not None:
            m.set_axon_ntff_profile_hook(hook)
    except Exception:
        pass


_install_ntff_shim()

import ml_dtypes  # noqa: E402

import concourse.mybir as mybir  # noqa: E402
import concourse.tile as tile  # noqa: E402
from concourse import bacc  # noqa: E402
from concourse.bass_utils import run_bass_kernel_spmd  # noqa: E402

F32 = mybir.dt.float32
BF16 = mybir.dt.bfloat16
BF = ml_dtypes.bfloat16

B = 2
T = 2048
D = 1024
NCORES = 8
NB = 8       # 256-token (u) blocks per batch
UB = 256     # tokens per block
NC8 = 8      # contraction chunks (d = 8*128)
NH = 4       # local heads per core

_CACHED = None
LAST_RESULTS = None


def _build_module():
    nc = bacc.Bacc("TRN2", target_bir_lowering=False, debug=False,
                   num_devices=NCORES)

    xt_d = nc.dram_tensor("xt", [NB, 128, NC8, UB], BF16,
                          kind="ExternalInput").ap()
    wv_d = nc.dram_tensor("wv", [128, NC8, 256], BF16,
                          kind="ExternalInput").ap()
    wo_d = nc.dram_tensor("wo", [128, 8, D], BF16, kind="ExternalInput").ap()
    out_d = nc.dram_tensor("out", [NH, 128, D], F32, kind="ExternalOutput").ap()

    with tile.TileContext(nc) as tc:
        _emit(nc, tc, xt_d, wv_d, wo_d, out_d)
    nc.compile()
    return nc


def _emit(nc, tc, xt_d, wv_d, wo_d, out_d):
    from contextlib import ExitStack

    ctx = ExitStack()
    with ctx:
        wpool = ctx.enter_context(tc.tile_pool(name="w", bufs=1))
        xtp = ctx.enter_context(tc.tile_pool(name="xt", bufs=NB))
        vtp = ctx.enter_context(tc.tile_pool(name="vt", bufs=1))
        outp = ctx.enter_context(tc.tile_pool(name="outsb", bufs=4))
        ps_v = ctx.enter_context(tc.tile_pool(name="ps_v", bufs=4, space="PSUM"))
        ps_o = ctx.enter_context(tc.tile_pool(name="ps_o", bufs=4, space="PSUM"))

        # wv in two tiles so the first v-matmuls only wait on the small one
        wva = wpool.tile([128, 4, 256], BF16, tag="wva")
        wvb = wpool.tile([128, 4, 256], BF16, tag="wvb")
        nc.scalar.dma_start(wva[:], wv_d[:, 0:4, :])
        nc.scalar.dma_start(wvb[:], wv_d[:, 4:8, :])

        def wv_lhs(c8, hp):
            t = wva if c8 < 4 else wvb
            return t[:, c8 % 4, 128 * hp:128 * hp + 128]

        wo_sb = wpool.tile([128, 8, D], BF16, tag="wo")
        # wo chunk m2 is first needed by the out-proj chunk of stream block
        # k=m2; ride the gpsimd queue so it doesn't delay wv/x
        for m2 in range(8):
            nc.gpsimd.dma_start(wo_sb[:, m2, :], wo_d[:, m2, :])

        # x token-blocks on the sync queue (block 0 gates PE start)
        xts = []
        for k in range(NB):
            xt = xtp.tile([128, NC8, UB], BF16, tag="xt", name=f"xt{k}")
            nc.sync.dma_start(xt[:], xt_d[k])
            xts.append(xt)

        # vt[h][64*(m%2)+di, 128*(m//2)+s] = v[t=16s+m, 256g+64h+di], bf16
        vt = [vtp.tile([128, D], BF16, tag=f"vt{h}", name=f"vt{h}")
              for h in range(NH)]

        psA = [ps_o.tile([128, 512], F32, tag="po", name=f"psA{h}")
               for h in range(NH)]

        # gpsimd cannot access PSUM, so evacuations go on DVE + Act only
        copy_engines = [nc.vector, nc.scalar]
        ce = [0]

        def ecopy(dst, src):
            eng = copy_engines[ce[0] % 2]
            ce[0] += 1
            if eng is nc.scalar:
                eng.copy(dst, src)
            else:
                eng.tensor_copy(dst, src)

        def vblock(k):
            psv = [ps_v.tile([128, UB], F32, tag="pv", name=f"pv{k}_{hp}")
                   for hp in range(2)]
            for c8 in range(NC8):
                for hp in range(2):
                    nc.tensor.matmul(psv[hp][:], wv_lhs(c8, hp),
                                     xts[k][:, c8, :],
                                     start=(c8 == 0), stop=(c8 == NC8 - 1))
            return psv

        def evac(k, psv):
            # block k holds m in {2k, 2k+1}; j = m%2 = local u//128
            for hp in range(2):
                for hh in range(2):
                    for j in range(2):
                        ecopy(vt[2 * hp + hh][64 * j:64 * j + 64,
                                              128 * k:128 * k + 128],
                              psv[hp][64 * hh:64 * hh + 64,
                                      128 * j:128 * j + 128])

        def outA(k):
            for h in range(NH):
                nc.tensor.matmul(psA[h][:], vt[h][:, 128 * k:128 * k + 128],
                                 wo_sb[:, k, 0:512],
                                 start=(k == 0), stop=(k == NB - 1))

        def flushA(h):
            ob = outp.tile([128, 512], F32, tag="ob", name=f"obA{h}")
            ecopy(ob[:], psA[h][:])
            nc.sync.dma_start(out_d[h, :, 0:512], ob[:])

        # stream: v-proj chases x DMAs; out-proj chunk k-1 fills PE slack
        psv_prev = vblock(0)
        evac(0, psv_prev)
        for k in range(1, NB):
            psv = vblock(k)
            evac(k, psv)
            outA(k - 1)
        outA(NB - 1)

        # second pass: out-proj columns [512,1024) + drains
        for h in range(NH):
            flushA(h)
            psB = ps_o.tile([128, 512], F32, tag="po", name=f"psB{h}")
            for m2 in range(8):
                nc.tensor.matmul(psB[:], vt[h][:, 128 * m2:128 * m2 + 128],
                                 wo_sb[:, m2, 512:1024],
                                 start=(m2 == 0), stop=(m2 == 7))
            ob = outp.tile([128, 512], F32, tag="ob", name=f"obB{h}")
            ecopy(ob[:], psB[:])
            nc.sync.dma_start(out_d[h, :, 512:1024], ob[:])


def _get_module():
    global _CACHED
    if _CACHED is None:
        _CACHED = _build_module()
    return _CACHED


def kernel(x, mask, Wq, Wk, Wv, Wo):
    global LAST_RESULTS
    x = np.asarray(x, dtype=np.float32)
    Wv = np.asarray(Wv, dtype=np.float32)
    Wo = np.asarray(Wo, dtype=np.float32)

    b, t, d = x.shape
    assert (b, t, d) == (B, T, D), (b, t, d)

    # x^T with tokens permuted to u = 128m + s (original t = 16s + m),
    # laid out [k, p, c8, u] to match the SBUF tiles exactly
    xts = []
    for bb in range(B):
        xT = x[bb].T                                      # [d, t]
        xTp = xT.reshape(D, 128, 16).transpose(0, 2, 1).reshape(D, T)
        xt = xTp.reshape(NC8, 128, NB, UB).transpose(2, 1, 0, 3)
        xts.append(np.ascontiguousarray(xt).astype(BF))

    # wv[p, c8, col] = Wv[128*c8 + p, col]; per-core slice of 256 cols
    wvp = Wv.reshape(NC8, 128, D).transpose(1, 0, 2)
    # wo[p, m2, n] = Wo.T[128*m2 + p, n]
    woT = np.ascontiguousarray(
        Wo.T.reshape(8, 128, D).transpose(1, 0, 2)).astype(BF)

    in_maps = []
    for c in range(NCORES):
        bb, g = c // 4, c % 4
        in_maps.append({
            "xt": xts[bb],
            "wv": np.ascontiguousarray(
                wvp[:, :, 256 * g:256 * g + 256]).astype(BF),
            "wo": woT,
        })

    nc = _get_module()
    res = run_bass_kernel_spmd(nc, in_maps, list(range(NCORES)))
    LAST_RESULTS = res

    out = np.empty((B, T, D), np.float32)
    for c in range(NCORES):
        bb, g = c // 4, c % 4
        out[bb, 512 * g:512 * g + 512, :] = \
            np.asarray(res.results[c]["out"], np.float32).reshape(512, D)
    return out


# revision 8
# speedup vs baseline: 1.8739x; 1.0700x over previous
"""nn_MultiHeadAttention_59253368815813 on 8 TRN2 NeuronCores.

The reference module is bug-faithful to its original nn.Module in two ways
that together collapse the computation:

  1. ``o = jnp.einsum('bhtl,bthd->bhtd', A, v)`` indexes ``v`` by the QUERY
     position ``t``, not the key position ``l``. ``l`` therefore only sums
     over the softmax weights, which sum to exactly 1 per row:
     ``o[b,h,t,d] == v[b,t,h,d]``. Q, K, the mask and the softmax never
     influence the output.
  2. ``o.reshape(b, T, d)`` with no transpose scrambles (head, token) so the
     reshaped activation row tj = 128*h + s is the concatenation over
     m=0..15 of v[b, 16*s+m, h, :].

So the exact computation is  out = scramble(x @ Wv) @ Wo.T,  and the
scramble makes output rows depend on one head only.

Sharding: 2 batches x 4 head-groups. Core c = (b=c//4, g=c%4) owns batch b
and heads {4g..4g+3} = Wv columns [256g, 256g+256) and output rows
[512g, 512g+512) of batch b. Each core loads only its batch's x (4.2MB in
bf16) instead of all of x, which is what made the previous version
DMA-bound (23.3MB/core at a shared ~360GB/s).

Per core, all in bf16 (PE runs bf16 at 1 cycle/row like f32r, but DMA
halves; quantization error ~2e-3 << the 2e-2 gate):
  stream x^T (tokens permuted to u = 128m + s, t = 16s + m) in 8 blocks of
  256 tokens; v-proj psum [128,256] per head-pair chases the stream; the
  reshape scramble happens in the psum->SBUF evacuation copies (spread over
  DVE/Pool/Act engines); output-projection columns [0,512) accumulate
  interleaved with the stream (chunk k uses only v tokens of block k);
  columns [512,1024) run as a second pass after the stream, overlapping the
  output DMAs.
"""

import sys
import types

import numpy as np

_TRN_REPO = "/opt/trn_rl_repo"
if _TRN_REPO not in sys.path:
    sys.path.insert(0, _TRN_REPO)


def _install_ntff_shim():
    """antenv.axon_hooks is absent in this container; provide it so
    BASS_TRACE=1 profiling works. No-op if the real module exists."""
    try:
        import antenv  # noqa: F401
    except ImportError:
        return
    if "antenv.axon_hooks" in sys.modules:
        return
    try:
        import antenv.axon_hooks  # noqa: F401
        return
    except ImportError:
        pass
    m = types.ModuleType("antenv.axon_hooks")
    m._hook = None
    m.set_axon_ntff_profile_hook = lambda h: setattr(m, "_hook", h)
    m.get_axon_ntff_profile_hook = lambda: m._hook
    sys.modules["antenv.axon_hooks"] = m
    try:
        from trn_agent_boot.trn_boot import _ntff_profile_via_ctypes

        hook = _ntff_profile_via_ctypes("/opt/axon/libaxon_pjrt.so")
        if hook is not None:
            m.set_axon_ntff_profile_hook(hook)
    except Exception:
        pass


_install_ntff_shim()

import ml_dtypes  # noqa: E402

import concourse.mybir as mybir  # noqa: E402
import concourse.tile as tile  # noqa: E402
from concourse import bacc  # noqa: E402
from concourse.bass_utils import run_bass_kernel_spmd  # noqa: E402

F32 = mybir.dt.float32
BF16 = mybir.dt.bfloat16
BF = ml_dtypes.bfloat16

B = 2
T = 2048
D = 1024
NCORES = 8
NB = 8       # 256-token (u) blocks per batch
UB = 256     # tokens per block
NC8 = 8      # contraction chunks (d = 8*128)
NH = 4       # local heads per core

_CACHED = None
LAST_RESULTS = None


def _build_module():
    nc = bacc.Bacc("TRN2", target_bir_lowering=False, debug=False,
                   num_devices=NCORES)

    xt_d = nc.dram_tensor("xt", [NB, 128, NC8, UB], BF16,
                          kind="ExternalInput").ap()
    wv_d = nc.dram_tensor("wv", [128, NC8, 256], BF16,
                          kind="ExternalInput").ap()
    wo_d = nc.dram_tensor("wo", [128, 8, D], BF16, kind="ExternalInput").ap()
    out_d = nc.dram_tensor("out", [NH, 128, D], F32, kind="ExternalOutput").ap()

    with tile.TileContext(nc) as tc:
        _emit(nc, tc, xt_d, wv_d, wo_d, out_d)
    nc.compile()
    return nc


def _emit(nc, tc, xt_d, wv_d, wo_d, out_d):
    from contextlib import ExitStack

    ctx = ExitStack()
    with ctx:
        wpool = ctx.enter_context(tc.tile_pool(name="w", bufs=1))
        xtp = ctx.enter_context(tc.tile_pool(name="xt", bufs=NB))
        vtp = ctx.enter_context(tc.tile_pool(name="vt", bufs=1))
        outp = ctx.enter_context(tc.tile_pool(name="outsb", bufs=4))
        ps_v = ctx.enter_context(tc.tile_pool(name="ps_v", bufs=4, space="PSUM"))
        ps_o = ctx.enter_context(tc.tile_pool(name="ps_o", bufs=4, space="PSUM"))

        # ALL input DMAs ride the sync queue in exact need-order: SP has no
        # preamble, and a single queue gives strict priority ordering so the
        # PE-gating transfers (wv chunks, x block 0) land first.
        wva = wpool.tile([128, 2, 256], BF16, tag="wva")
        wvb = wpool.tile([128, 2, 256], BF16, tag="wvb")
        wvc = wpool.tile([128, 4, 256], BF16, tag="wvc")

        def wv_lhs(c8, hp):
            t, i = (wva, c8) if c8 < 2 else (wvb, c8 - 2) if c8 < 4 \
                else (wvc, c8 - 4)
            return t[:, i, 128 * hp:128 * hp + 128]

        wo_sb = wpool.tile([128, 8, D], BF16, tag="wo")
        xts = [xtp.tile([128, NC8, UB], BF16, tag="xt", name=f"xt{k}")
               for k in range(NB)]

        nc.sync.dma_start(wva[:], wv_d[:, 0:2, :])
        nc.sync.dma_start(xts[0][:], xt_d[0])
        nc.sync.dma_start(wvb[:], wv_d[:, 2:4, :])
        nc.sync.dma_start(wvc[:], wv_d[:, 4:8, :])
        nc.sync.dma_start(xts[1][:], xt_d[1])
        # x block k+2 and wo chunk k are both first needed while the PE
        # works on block k+1; keep x one slot ahead of wo
        for k in range(NB - 2):
            nc.sync.dma_start(xts[k + 2][:], xt_d[k + 2])
            nc.sync.dma_start(wo_sb[:, k, :], wo_d[:, k, :])
        for m2 in range(NB - 2, 8):
            nc.sync.dma_start(wo_sb[:, m2, :], wo_d[:, m2, :])

        # vt[h][64*(m%2)+di, 128*(m//2)+s] = v[t=16s+m, 256g+64h+di], bf16
        vt = [vtp.tile([128, D], BF16, tag=f"vt{h}", name=f"vt{h}")
              for h in range(NH)]

        psA = [ps_o.tile([128, 512], F32, tag="po", name=f"psA{h}")
               for h in range(NH)]

        # gpsimd cannot access PSUM, so evacuations go on DVE + Act only
        copy_engines = [nc.vector, nc.scalar]
        ce = [0]

        def ecopy(dst, src):
            eng = copy_engines[ce[0] % 2]
            ce[0] += 1
            if eng is nc.scalar:
                eng.copy(dst, src)
            else:
                eng.tensor_copy(dst, src)

        def vblock(k):
            psv = [ps_v.tile([128, UB], F32, tag="pv", name=f"pv{k}_{hp}")
                   for hp in range(2)]
            for c8 in range(NC8):
                for hp in range(2):
                    nc.tensor.matmul(psv[hp][:], wv_lhs(c8, hp),
                                     xts[k][:, c8, :],
                                     start=(c8 == 0), stop=(c8 == NC8 - 1))
            return psv

        def evac(k, psv):
            # block k holds m in {2k, 2k+1}; j = m%2 = local u//128
            for hp in range(2):
                for hh in range(2):
                    for j in range(2):
                        ecopy(vt[2 * hp + hh][64 * j:64 * j + 64,
                                              128 * k:128 * k + 128],
                              psv[hp][64 * hh:64 * hh + 64,
                                      128 * j:128 * j + 128])

        def outA(k):
            for h in range(NH):
                nc.tensor.matmul(psA[h][:], vt[h][:, 128 * k:128 * k + 128],
                                 wo_sb[:, k, 0:512],
                                 start=(k == 0), stop=(k == NB - 1))

        def flushA(h):
            ob = outp.tile([128, 512], F32, tag="ob", name=f"obA{h}")
            ecopy(ob[:], psA[h][:])
            nc.gpsimd.dma_start(out_d[h, :, 0:512], ob[:])

        # stream: v-proj chases x DMAs; out-proj chunk k-1 fills PE slack
        psv_prev = vblock(0)
        evac(0, psv_prev)
        for k in range(1, NB):
            psv = vblock(k)
            evac(k, psv)
            outA(k - 1)
        outA(NB - 1)

        # second pass: out-proj columns [512,1024) + drains
        for h in range(NH):
            flushA(h)
            psB = ps_o.tile([128, 512], F32, tag="po", name=f"psB{h}")
            for m2 in range(8):
                nc.tensor.matmul(psB[:], vt[h][:, 128 * m2:128 * m2 + 128],
                                 wo_sb[:, m2, 512:1024],
                                 start=(m2 == 0), stop=(m2 == 7))
            ob = outp.tile([128, 512], F32, tag="ob", name=f"obB{h}")
            ecopy(ob[:], psB[:])
            nc.gpsimd.dma_start(out_d[h, :, 512:1024], ob[:])


def _get_module():
    global _CACHED
    if _CACHED is None:
        _CACHED = _build_module()
    return _CACHED


def kernel(x, mask, Wq, Wk, Wv, Wo):
    global LAST_RESULTS
    x = np.asarray(x, dtype=np.float32)
    Wv = np.asarray(Wv, dtype=np.float32)
    Wo = np.asarray(Wo, dtype=np.float32)

    b, t, d = x.shape
    assert (b, t, d) == (B, T, D), (b, t, d)

    # x^T with tokens permuted to u = 128m + s (original t = 16s + m),
    # laid out [k, p, c8, u] to match the SBUF tiles exactly
    xts = []
    for bb in range(B):
        xT = x[bb].T                                      # [d, t]
        xTp = xT.reshape(D, 128, 16).transpose(0, 2, 1).reshape(D, T)
        xt = xTp.reshape(NC8, 128, NB, UB).transpose(2, 1, 0, 3)
        xts.append(np.ascontiguousarray(xt).astype(BF))

    # wv[p, c8, col] = Wv[128*c8 + p, col]; per-core slice of 256 cols
    wvp = Wv.reshape(NC8, 128, D).transpose(1, 0, 2)
    # wo[p, m2, n] = Wo.T[128*m2 + p, n]
    woT = np.ascontiguousarray(
        Wo.T.reshape(8, 128, D).transpose(1, 0, 2)).astype(BF)

    in_maps = []
    for c in range(NCORES):
        bb, g = c // 4, c % 4
        in_maps.append({
            "xt": xts[bb],
            "wv": np.ascontiguousarray(
                wvp[:, :, 256 * g:256 * g + 256]).astype(BF),
            "wo": woT,
        })

    nc = _get_module()
    res = run_bass_kernel_spmd(nc, in_maps, list(range(NCORES)))
    LAST_RESULTS = res

    out = np.empty((B, T, D), np.float32)
    for c in range(NCORES):
        bb, g = c // 4, c % 4
        out[bb, 512 * g:512 * g + 512, :] = \
            np.asarray(res.results[c]["out"], np.float32).reshape(512, D)
    return out


# revision 13
# speedup vs baseline: 1.9706x; 1.0516x over previous
"""nn_MultiHeadAttention_59253368815813 on 8 TRN2 NeuronCores.

The reference module is bug-faithful to its original nn.Module in two ways
that together collapse the computation:

  1. ``o = jnp.einsum('bhtl,bthd->bhtd', A, v)`` indexes ``v`` by the QUERY
     position ``t``, not the key position ``l``. ``l`` therefore only sums
     over the softmax weights, which sum to exactly 1 per row:
     ``o[b,h,t,d] == v[b,t,h,d]``. Q, K, the mask and the softmax never
     influence the output.
  2. ``o.reshape(b, T, d)`` with no transpose scrambles (head, token) so the
     reshaped activation row tj = 128*h + s is the concatenation over
     m=0..15 of v[b, 16*s+m, h, :].

So the exact computation is  out = scramble(x @ Wv) @ Wo.T,  and the
scramble makes output rows depend on one head only.

Sharding: 2 batches x 4 head-groups. Core c = (b=c//4, g=c%4) owns batch b
and heads {4g..4g+3} = Wv columns [256g, 256g+256) and output rows
[512g, 512g+512) of batch b. Each core loads only its batch's x (4.2MB in
bf16) instead of all of x, which is what made the previous version
DMA-bound (23.3MB/core at a shared ~360GB/s).

Per core, all in bf16 (PE runs bf16 at 1 cycle/row like f32r, but DMA
halves; quantization error ~2e-3 << the 2e-2 gate):
  stream x^T (tokens permuted to u = 128m + s, t = 16s + m) in 8 blocks of
  256 tokens; v-proj psum [128,256] per head-pair chases the stream; the
  reshape scramble happens in the psum->SBUF evacuation copies (spread over
  DVE/Pool/Act engines); output-projection columns [0,512) accumulate
  interleaved with the stream (chunk k uses only v tokens of block k);
  columns [512,1024) run as a second pass after the stream, overlapping the
  output DMAs.
"""

import sys
import types

import numpy as np

_TRN_REPO = "/opt/trn_rl_repo"
if _TRN_REPO not in sys.path:
    sys.path.insert(0, _TRN_REPO)


def _install_ntff_shim():
    """antenv.axon_hooks is absent in this container; provide it so
    BASS_TRACE=1 profiling works. No-op if the real module exists."""
    try:
        import antenv  # noqa: F401
    except ImportError:
        return
    if "antenv.axon_hooks" in sys.modules:
        return
    try:
        import antenv.axon_hooks  # noqa: F401
        return
    except ImportError:
        pass
    m = types.ModuleType("antenv.axon_hooks")
    m._hook = None
    m.set_axon_ntff_profile_hook = lambda h: setattr(m, "_hook", h)
    m.get_axon_ntff_profile_hook = lambda: m._hook
    sys.modules["antenv.axon_hooks"] = m
    try:
        from trn_agent_boot.trn_boot import _ntff_profile_via_ctypes

        hook = _ntff_profile_via_ctypes("/opt/axon/libaxon_pjrt.so")
        if hook is not None:
            m.set_axon_ntff_profile_hook(hook)
    except Exception:
        pass


_install_ntff_shim()

import ml_dtypes  # noqa: E402

import concourse.mybir as mybir  # noqa: E402
import concourse.tile as tile  # noqa: E402
from concourse import bacc  # noqa: E402
from concourse.bass_utils import run_bass_kernel_spmd  # noqa: E402

F32 = mybir.dt.float32
BF16 = mybir.dt.bfloat16
BF = ml_dtypes.bfloat16

B = 2
T = 2048
D = 1024
NCORES = 8
NB = 8       # 256-token (u) blocks per batch
UB = 256     # tokens per block
NC8 = 8      # contraction chunks (d = 8*128)
NH = 4       # local heads per core

_CACHED = None
LAST_RESULTS = None


def _build_module():
    nc = bacc.Bacc("TRN2", target_bir_lowering=False, debug=False,
                   num_devices=NCORES)

    xt_d = nc.dram_tensor("xt", [NB, 128, NC8, UB], BF16,
                          kind="ExternalInput").ap()
    wv_d = nc.dram_tensor("wv", [128, NC8, 256], BF16,
                          kind="ExternalInput").ap()
    wo_d = nc.dram_tensor("wo", [128, 8, D], BF16, kind="ExternalInput").ap()
    out_d = nc.dram_tensor("out", [NH, 128, D], BF16,
                           kind="ExternalOutput").ap()

    with tile.TileContext(nc) as tc:
        _emit(nc, tc, xt_d, wv_d, wo_d, out_d)
    nc.compile()
    return nc


def _emit(nc, tc, xt_d, wv_d, wo_d, out_d):
    from contextlib import ExitStack

    ctx = ExitStack()
    with ctx:
        wpool = ctx.enter_context(tc.tile_pool(name="w", bufs=1))
        xtp = ctx.enter_context(tc.tile_pool(name="xt", bufs=NB))
        vtp = ctx.enter_context(tc.tile_pool(name="vt", bufs=1))
        outp = ctx.enter_context(tc.tile_pool(name="outsb", bufs=4))
        ps_v = ctx.enter_context(tc.tile_pool(name="ps_v", bufs=4, space="PSUM"))
        ps_o = ctx.enter_context(tc.tile_pool(name="ps_o", bufs=4, space="PSUM"))

        # ALL input DMAs ride the sync queue in exact need-order: SP has no
        # preamble, and a single queue gives strict priority ordering so the
        # PE-gating transfers (wv chunks, x block 0) land first.
        wva = wpool.tile([128, 2, 256], BF16, tag="wva")
        wvb = wpool.tile([128, 2, 256], BF16, tag="wvb")
        wvc = wpool.tile([128, 4, 256], BF16, tag="wvc")

        def wv_lhs(c8, hp):
            t, i = (wva, c8) if c8 < 2 else (wvb, c8 - 2) if c8 < 4 \
                else (wvc, c8 - 4)
            return t[:, i, 128 * hp:128 * hp + 128]

        wo_sb = wpool.tile([128, 8, D], BF16, tag="wo")
        # block 0 is split in half-tiles so the first v-matmuls start after
        # only half of it (plus wva) has landed
        xt0a = xtp.tile([128, 4, UB], BF16, tag="xt0a")
        xt0b = xtp.tile([128, 4, UB], BF16, tag="xt0b")
        xts = [None] + [xtp.tile([128, NC8, UB], BF16, tag="xt",
                                 name=f"xt{k}") for k in range(1, NB)]

        def x_rhs(k, c8):
            if k == 0:
                t = xt0a if c8 < 4 else xt0b
                return t[:, c8 % 4, :]
            return xts[k][:, c8, :]

        nc.sync.dma_start(wva[:], wv_d[:, 0:2, :])
        nc.sync.dma_start(xt0a[:], xt_d[0, :, 0:4, :])
        nc.sync.dma_start(wvb[:], wv_d[:, 2:4, :])
        nc.sync.dma_start(xt0b[:], xt_d[0, :, 4:8, :])
        nc.sync.dma_start(wvc[:], wv_d[:, 4:8, :])
        nc.sync.dma_start(xts[1][:], xt_d[1])
        nc.sync.dma_start(xts[2][:], xt_d[2])
        # x block k+3 and wo chunk k: x keeps a two-slot lead over wo
        for k in range(NB - 3):
            nc.sync.dma_start(xts[k + 3][:], xt_d[k + 3])
            nc.sync.dma_start(wo_sb[:, k, :], wo_d[:, k, :])
        for m2 in range(NB - 3, 8):
            nc.sync.dma_start(wo_sb[:, m2, :], wo_d[:, m2, :])

        # vt[h][64*(m%2)+di, 128*(m//2)+s] = v[t=16s+m, 256g+64h+di], bf16
        vt = [vtp.tile([128, D], BF16, tag=f"vt{h}", name=f"vt{h}")
              for h in range(NH)]

        psA = [ps_o.tile([128, 512], F32, tag="po", name=f"psA{h}")
               for h in range(NH)]

        # gpsimd cannot access PSUM, so evacuations go on DVE + Act only
        copy_engines = [nc.vector, nc.scalar]
        ce = [0]

        def ecopy(dst, src):
            eng = copy_engines[ce[0] % 2]
            ce[0] += 1
            if eng is nc.scalar:
                eng.copy(dst, src)
            else:
                eng.tensor_copy(dst, src)

        def vblock(k):
            psv = [ps_v.tile([128, UB], F32, tag="pv", name=f"pv{k}_{hp}")
                   for hp in range(2)]
            for c8 in range(NC8):
                for hp in range(2):
                    nc.tensor.matmul(psv[hp][:], wv_lhs(c8, hp),
                                     x_rhs(k, c8),
                                     start=(c8 == 0), stop=(c8 == NC8 - 1))
            return psv

        def evac(k, psv):
            # block k holds m in {2k, 2k+1}; j = m%2 = local u//128
            for hp in range(2):
                for hh in range(2):
                    for j in range(2):
                        ecopy(vt[2 * hp + hh][64 * j:64 * j + 64,
                                              128 * k:128 * k + 128],
                              psv[hp][64 * hh:64 * hh + 64,
                                      128 * j:128 * j + 128])

        def outA(k):
            for h in range(NH):
                nc.tensor.matmul(psA[h][:], vt[h][:, 128 * k:128 * k + 128],
                                 wo_sb[:, k, 0:512],
                                 start=(k == 0), stop=(k == NB - 1))

        def flushA(h):
            ob = outp.tile([128, 512], BF16, tag="ob", name=f"obA{h}")
            ecopy(ob[:], psA[h][:])
            nc.scalar.dma_start(out_d[h, :, 0:512], ob[:])

        # stream: v-proj chases x DMAs; out-proj chunk k-1 fills PE slack
        psv_prev = vblock(0)
        evac(0, psv_prev)
        for k in range(1, NB):
            psv = vblock(k)
            evac(k, psv)
            outA(k - 1)
        outA(NB - 1)

        # queue all psA evacuations first so the psB bank-reuse waits clear
        # while the first psB groups are still accumulating
        for h in range(NH):
            flushA(h)

        # second pass: out-proj columns [512,1024) + drains
        for h in range(NH):
            psB = ps_o.tile([128, 512], F32, tag="po", name=f"psB{h}")
            for m2 in range(8):
                nc.tensor.matmul(psB[:], vt[h][:, 128 * m2:128 * m2 + 128],
                                 wo_sb[:, m2, 512:1024],
                                 start=(m2 == 0), stop=(m2 == 7))
            ob = outp.tile([128, 512], BF16, tag="ob", name=f"obB{h}")
            ecopy(ob[:], psB[:])
            nc.scalar.dma_start(out_d[h, :, 512:1024], ob[:])


def _get_module():
    global _CACHED
    if _CACHED is None:
        _CACHED = _build_module()
    return _CACHED


def kernel(x, mask, Wq, Wk, Wv, Wo):
    global LAST_RESULTS
    x = np.asarray(x, dtype=np.float32)
    Wv = np.asarray(Wv, dtype=np.float32)
    Wo = np.asarray(Wo, dtype=np.float32)

    b, t, d = x.shape
    assert (b, t, d) == (B, T, D), (b, t, d)

    # x^T with tokens permuted to u = 128m + s (original t = 16s + m),
    # laid out [k, p, c8, u] to match the SBUF tiles exactly
    xts = []
    for bb in range(B):
        xT = x[bb].T                                      # [d, t]
        xTp = xT.reshape(D, 128, 16).transpose(0, 2, 1).reshape(D, T)
        xt = xTp.reshape(NC8, 128, NB, UB).transpose(2, 1, 0, 3)
        xts.append(np.ascontiguousarray(xt).astype(BF))

    # wv[p, c8, col] = Wv[128*c8 + p, col]; per-core slice of 256 cols
    wvp = Wv.reshape(NC8, 128, D).transpose(1, 0, 2)
    # wo[p, m2, n] = Wo.T[128*m2 + p, n]
    woT = np.ascontiguousarray(
        Wo.T.reshape(8, 128, D).transpose(1, 0, 2)).astype(BF)

    in_maps = []
    for c in range(NCORES):
        bb, g = c // 4, c % 4
        in_maps.append({
            "xt": xts[bb],
            "wv": np.ascontiguousarray(
                wvp[:, :, 256 * g:256 * g + 256]).astype(BF),
            "wo": woT,
        })

    nc = _get_module()
    res = run_bass_kernel_spmd(nc, in_maps, list(range(NCORES)))
    LAST_RESULTS = res

    out = np.empty((B, T, D), np.float32)
    for c in range(NCORES):
        bb, g = c // 4, c % 4
        out[bb, 512 * g:512 * g + 512, :] = \
            np.asarray(res.results[c]["out"]).astype(np.float32).reshape(512, D)
    return out
